# revision 1
# baseline (speedup 1.0000x reference)
"""ALiBi (attention linear biases) kernel for Trainium2, 8 NeuronCores.

Problem: out = attention_scores + bias, where
  attention_scores: (2, 16, 2048, 2048) f32
  bias[h, j] = slopes[h] * (j - 2047)  (causal ALiBi row bias, broadcast
  over batch and query rows)

Sharding: 2 batches x 16 heads = 32 (batch, head) matrices, 4 per core
across 8 cores. Each core processes a (8192, 2048) f32 slab: tiled DMA
load -> vector add of a per-head bias row (pre-broadcast across the 128
partitions) -> DMA store. Memory-bound: ~134 MB HBM traffic per core.
"""

import os
import sys

import numpy as np

# Defensive: make sure the concourse/axon stack resolves even if the
# grading environment lacks the usual PYTHONPATH entries.
for _p in (
    "/root/.axon_site",
    "/root/.axon_site/_ro/trn_rl_repo",
    "/root/.axon_site/_ro/pypackages",
    "/opt/trn_rl_repo",
):
    if os.path.isdir(_p) and _p not in sys.path:
        sys.path.append(_p)
os.environ.setdefault("JAX_PLATFORMS", "axon,cpu")

NUM_HEADS = 16
SEQ = 2048
BATCH = 2
N_CORES = 8
PAIRS = BATCH * NUM_HEADS            # 32 (batch, head) matrices
PAIRS_PER_CORE = PAIRS // N_CORES    # 4
ROWS_PER_CORE = PAIRS_PER_CORE * SEQ # 8192
P = 128                              # SBUF partitions
ROWS_PER_PART = 4                    # rows folded into the free dim
TILE_ROWS = P * ROWS_PER_PART        # 512 rows per tile (4 MiB)
TILES_PER_PAIR = SEQ // TILE_ROWS    # 4
TILES_PER_CORE = ROWS_PER_CORE // TILE_ROWS  # 16
DATA_BUFS = 4
# When True, the host sends bias rows (4, 2048) = 32 KB and the device
# broadcasts across partitions via a ones-matmul; when False, the host
# sends the pre-broadcast (4, 128, 2048) = 4 MiB.
BIAS_ON_DEVICE = False

_NC_CACHE = None


def _build_nc(rows_per_part=ROWS_PER_PART, bufs=DATA_BUFS, load_eng="sync",
              store_eng="scalar", alternate=True, repeat=1, split_halves=False,
              split_free=False):
    import concourse.bacc as bacc
    import concourse.mybir as mybir
    from concourse.tile import TileContext

    f32 = mybir.dt.float32
    # Bacc (not raw Bass): its compile() splits multi-sem waits into event
    # semaphores — TRN2 allows at most one sync wait per engine instruction.
    nc = bacc.Bacc()
    bias_dev = BIAS_ON_DEVICE
    scores = nc.declare_dram_parameter(
        "scores", [ROWS_PER_CORE, SEQ], f32, isOutput=False
    )
    bias_shape = [PAIRS_PER_CORE, SEQ] if bias_dev else [PAIRS_PER_CORE, P, SEQ]
    bias = nc.declare_dram_parameter("bias", bias_shape, f32, isOutput=False)
    out = nc.declare_dram_parameter("out", [ROWS_PER_CORE, SEQ], f32, isOutput=True)

    tile_rows = P * rows_per_part
    tiles_per_pair = SEQ // tile_rows
    n_tiles = ROWS_PER_CORE // tile_rows
    engines = {"sync": nc.sync, "scalar": nc.scalar, "gpsimd": nc.gpsimd,
               "vector": nc.vector, "pool": getattr(nc, "pool", nc.gpsimd)}

    # Partition p of tile t holds rows t*tile_rows + p*rows_per_part ..
    # -> each partition reads a contiguous span from HBM; the whole tile
    # is one contiguous block.
    scores_v = scores.rearrange("(t p n) m -> t p (n m)", p=P, n=rows_per_part)
    out_v = out.rearrange("(t p n) m -> t p (n m)", p=P, n=rows_per_part)

    with TileContext(nc) as tc:
        with (
            tc.tile_pool(name="bias", bufs=1) as bias_pool,
            tc.tile_pool(name="data", bufs=bufs) as pool,
        ):
            bias_tiles = []
            if bias_dev:
                with (
                    tc.tile_pool(name="brow", bufs=1) as brow_pool,
                    tc.tile_pool(name="bpsum", bufs=2, space="PSUM") as psum_pool,
                ):
                    ones = brow_pool.tile([1, P], f32, tag="ones")
                    nc.gpsimd.memset(ones[:], 1.0)
                    for q in range(PAIRS_PER_CORE):
                        row = brow_pool.tile([1, SEQ], f32, tag=f"row{q}")
                        nc.gpsimd.dma_start(out=row[:], in_=bias[q : q + 1])
                        bt = bias_pool.tile([P, SEQ], f32, tag=f"bias{q}")
                        ps = psum_pool.tile([P, SEQ], f32, tag="ps")
                        for j in range(SEQ // 512):
                            nc.tensor.matmul(
                                ps[:, j * 512 : (j + 1) * 512],
                                ones[:],
                                row[0:1, j * 512 : (j + 1) * 512],
                            )
                        nc.vector.tensor_copy(out=bt[:], in_=ps[:])
                        bias_tiles.append(bt)
            else:
                for q in range(PAIRS_PER_CORE):
                    bt = bias_pool.tile([P, SEQ], f32, tag=f"bias{q}")
                    # gpsimd (SWDGE): keeps the bias prologue off the two
                    # HWDGE rings so it overlaps the first data loads.
                    nc.gpsimd.dma_start(out=bt[:], in_=bias[q])
                    bias_tiles.append(bt)
            H = P // 2
            for rep in range(repeat):
                for t in range(n_tiles):
                    q = t // tiles_per_pair
                    if alternate and t % 2 == 1:
                        ld, st = engines[store_eng], engines[load_eng]
                    else:
                        ld, st = engines[load_eng], engines[store_eng]
                    tile = pool.tile([P, rows_per_part * SEQ], f32, tag="data")
                    F2 = rows_per_part * SEQ // 2
                    if split_halves:
                        # Each ring moves one contiguous partition-half of
                        # every tile; rings swap halves between load/store.
                        ld.dma_start(out=tile[:H], in_=scores_v[t][:H])
                        st.dma_start(out=tile[H:], in_=scores_v[t][H:])
                    elif split_free:
                        # Free-dim halves: both rings active on every tile at
                        # full 128-partition port width (16 KB per partition).
                        ld.dma_start(out=tile[:, :F2], in_=scores_v[t][:, :F2])
                        st.dma_start(out=tile[:, F2:], in_=scores_v[t][:, F2:])
                    else:
                        ld.dma_start(out=tile[:], in_=scores_v[t])
                    for k in range(rows_per_part):
                        nc.vector.tensor_add(
                            out=tile[:, k * SEQ : (k + 1) * SEQ],
                            in0=tile[:, k * SEQ : (k + 1) * SEQ],
                            in1=bias_tiles[q][:],
                        )
                    if split_halves:
                        st.dma_start(out=out_v[t][:H], in_=tile[:H])
                        ld.dma_start(out=out_v[t][H:], in_=tile[H:])
                    elif split_free:
                        st.dma_start(out=out_v[t][:, :F2], in_=tile[:, :F2])
                        ld.dma_start(out=out_v[t][:, F2:], in_=tile[:, F2:])
                    else:
                        st.dma_start(out=out_v[t], in_=tile[:])
    nc.compile()
    return nc


def _get_nc():
    global _NC_CACHE
    if _NC_CACHE is None:
        _NC_CACHE = _build_nc()
    return _NC_CACHE


def _alibi_bias_rows():
    """(NUM_HEADS, SEQ) f32: slopes[h] * (j - (SEQ-1)), matching reference."""
    ratio = 2.0 ** (-8.0 / NUM_HEADS)
    slopes = (ratio ** np.arange(1, 1 + NUM_HEADS, dtype=np.float64)).astype(
        np.float32
    )
    dist = np.arange(1 - SEQ, 1, dtype=np.float32)
    return slopes[:, None] * dist[None, :]


def _make_in_maps(attention_scores):
    x = np.ascontiguousarray(np.asarray(attention_scores), dtype=np.float32)
    assert x.shape == (BATCH, NUM_HEADS, SEQ, SEQ), x.shape
    flat = x.reshape(PAIRS, SEQ, SEQ)
    bias16 = _alibi_bias_rows()
    in_maps = []
    for c in range(N_CORES):
        lo = c * PAIRS_PER_CORE
        scores_c = flat[lo : lo + PAIRS_PER_CORE].reshape(ROWS_PER_CORE, SEQ)
        heads = [(lo + q) % NUM_HEADS for q in range(PAIRS_PER_CORE)]
        if BIAS_ON_DEVICE:
            bias_c = np.ascontiguousarray(bias16[heads], dtype=np.float32)
        else:
            bias_c = np.ascontiguousarray(
                np.broadcast_to(
                    bias16[heads][:, None, :], (PAIRS_PER_CORE, P, SEQ)
                ),
                dtype=np.float32,
            )
        in_maps.append({"scores": np.ascontiguousarray(scores_c), "bias": bias_c})
    return in_maps


def _run(in_maps, **kwargs):
    from concourse.bass_utils import run_bass_kernel_spmd

    return run_bass_kernel_spmd(
        _get_nc(), in_maps, core_ids=list(range(N_CORES)), **kwargs
    )


def _gather(results):
    out = np.concatenate(
        [np.asarray(r["out"]).reshape(PAIRS_PER_CORE, SEQ, SEQ) for r in results],
        axis=0,
    )
    return out.reshape(BATCH, NUM_HEADS, SEQ, SEQ)


def kernel(attention_scores):
    res = _run(_make_in_maps(attention_scores))
    return _gather(res.results)

